# revision 40
# baseline (speedup 1.0000x reference)
"""DigitCaps (CapsNet dynamic routing) Trainium2 kernel, v3.

v3 changes (193us -> ~185us, rel err 3.9e-3; VectorE busy 152 -> ~146us):
  - act-table pin: Sqrt -> exp(0.5*ln(x)) and a monkeypatch restricting
    Exp/Ln/Square/Copy/Identity to the single set
    natural_log_exp_and_others. ACT_TABLE_LOADs: 10x1.3us -> 1.
    (Plain Ln+Exp WITHOUT the pin made it WORSE - 19 loads - because the
    pass maps Ln to natural_log which lacks Exp.)
  - rzb CAST folded into cCb: one TT with broadcast rz (1x) replaces
    rzb-copy(1x)+cCb(2x): -0.5us/col.
  - bl and msks in bf16 (tree writes/accumulates at 2x).
  - exp-product: it1 bl-updates write their delta via the cheap
    first=True path; it2 uses ee2 = exp(delta)*ee1 (persistent ee1 per
    bg in eepool). Drops t1+bl-add TTs.
  - call build 2 lanes VectorE tensor_scalar + 6 ScalarE masked copies
    (all-Act delays s-matmuls +3us; all-DVE loads the binding engine).
  - s0 BEFORE einsum (needs only x2d+wr); vrep0 select-matmuls emitted
    between einsum xc-chunks (emitting them right after s0 stalled PE
    ~5us waiting on the squash chain). x2d DMA split in halves.
  MEASURED DEAD ENDS v3: PE warm-up dummy matmuls (p-state model wrong on
  HW - real gate is ~50% power throttle, util limit 0.7 avg); DVE TT/STT
  reading PSUM twice (walrus verifier rejects); gpsimd.tensor_copy from
  PSUM (verifier rejects); ps_pool bufs=3 (PSUM banks exhausted);
  tensor_scalar 4x mode (cost model says 4x_2p, HW runs 2x: 363ns for
  720 elems); TensorReduce has NO fast modes (1x) - the TT pairwise tree
  beats it everywhere.
  Remaining wall (185us): ~22 startup (launch 7 + s0+squash chain ~14,
  einsum pace ~94ns/matmul-slot under throttle), ~146 VectorE-dense
  (z 50 + t8 26 + t4 14 + t2 9 + softmax/call ~20 + stage3 ~8 + iter0
  copies/casts ~5 + misc), ~12 scattered sub-us sem-latency gaps, ~5
  tail (last s-matmul + extract + out-DMA flush).

Previous docstring (v2):

Math (per reference):
  u_hat[b,i,o,d] = sum_k W[i,o,d,k] * x[b,i,k]      B=256, IC=1152, K=8, O=10, D=16
  3 routing iters: c = softmax_o(bl); s = sum_i c*u_hat; v = squash(s);
                   bl += sum_d u_hat*v
  out v: [B, 10, 16]

Data-parallel over batch: 8 cores x 32 samples, 4 bgroups of 8 per core.
Einsum on TensorE with block-diagonal x (lhsT stationary, wr moving).
v2 design (326us baseline -> 198us, rel err 1.8e-2 -> 3.9e-3):
  - iter-0 s is a DENSE matmul: c uniform -> s0 = 0.1*sum_i u_hat
    = accumulation of x2d[g].T @ wr[g] over all 72 groups (out [32,160],
    f32 PSUM accumulate - also the big accuracy win). Removes 288
    routing matmuls + 4 diag extracts; v0 broadcast to (i16,b8) rows
    via a tiny select matmul per bgroup.
  - bl-update (the VectorE-bound core: ~15us per (iter,bgroup)):
    z = uh*vrep (bf16 2x mode) then pairwise d-tree, 2 chunks of 36
    groups; last tree level writes/accumulates bl directly.
  - softmax: exp->bf16, reciprocal_approx_fast for 1/Z, cC mult
    outputs bf16 directly (cast folded).
  - call build split: b' 0:4 on VectorE (one broadcast TT), b' 4:8 on
    ScalarE (per-partition-scalar masked copies) - keeps the critical
    VectorE queue short. The LAST column builds all 8 b' on VectorE
    (vcall_all) so no ScalarE hop sits on the exposed tail chain
    (trace-verified: tail idle 9.4us -> 5.9us).
  - software-pipelined emission across (iter,bgroup) columns: stage1
    (softmax+call), stage2 (s-matmul, TensorE), stage3 (extract+squash
    +vrep), bl-update lagged per upd_at (lag 2; last one lag 3 to cover
    the final it2 s-matmuls); first it1 column interleaved into iter-0;
    bg0's einsum copies alternate ScalarE/VectorE (V idle at start).
  - wr in 6 per-slice tiles so early matmuls don't wait the full DMA.
  Wall accounting (195us median, 193-198 device band): ~162us VectorE
  at the 2-elem/cyc bf16 ceiling (the bl-update d-contraction cannot
  leave DVE: PE contracts over partitions only - every reformulation
  hits a b-pairing or o-diag-extraction wall; ScalarE has no 2-tensor
  op), ~20us startup (8us launch + 5.9MB critical set at ~305GB/s DMA
  roofline), ~6us tail (exp + final s-matmul; split-accumulation for
  early extract measured worse twice).
  MEASURED DEAD ENDS - do not retry without new hardware facts:
  GpSimd tensor_tensor (4-8x slower than DVE, poisons pipeline; tried
  call halves, tree tails, full bl-update chunks); gpsimd.dma_start
  (routes to slow software-DGE); fp8 s-matmul (s ~= sum of cancelling
  terms so quant error hits s at full 3.6%, not /sqrt(N)); multi-dim
  strided stationary AP for LDWEIGHTS (compiler rejects); d-major t2
  tree level (strided write penalty); uh half/quarter tiles; s0-first
  T-queue order; deeper bl-update lag shuffles.
"""

import sys

sys.path.insert(0, "/opt/trn_rl_repo")

import numpy as np
import ml_dtypes

import concourse.bass as bass
import concourse.bacc as bacc_mod
from concourse import mybir
from concourse.tile import TileContext
from concourse.bass_utils import run_bass_kernel_spmd

# Pin every Act func we use (Exp/Ln/Square/Copy/Identity) to the single
# covering table set `natural_log_exp_and_others` so the act-table pass
# can never alternate sets (each ACT_TABLE_LOAD costs 1.3us on ScalarE;
# baseline thrashes 10-19 of them). Set keys/order are preserved so the
# act_func_set_id -> act_info.json index mapping stays valid.
_ONE_SET = "natural_log_exp_and_others"


def _pin_act_tables():
    import concourse.hw_specs as hw_specs

    orig = hw_specs.get_activation_tables
    if getattr(orig, "_digitcaps_pinned", False):
        return
    _pin = {"Exp", "Ln", "Square", "Copy", "Identity"}

    def patched(module_arch):
        t = orig(module_arch)
        out = {}
        for k, v in t.items():
            if k == _ONE_SET:
                out[k] = v
            else:
                out[k] = {f for f in v if f.name not in _pin}
        return out

    patched._digitcaps_pinned = True
    hw_specs.get_activation_tables = patched
    bacc_mod.get_activation_tables = patched


_pin_act_tables()

BF16 = ml_dtypes.bfloat16

# Problem dims (hardcoded per harness contract)
B, IC, KD, OC, OD = 256, 1152, 8, 10, 16
NCORES = 8
BL = B // NCORES          # 32 samples per core
BG = 8                    # bgroup size
NBG = BL // BG            # 4 bgroups
G = IC // 16              # 72 groups of 16 in-caps
ODF = OC * OD             # 160
ITERS = 3
GO = G * OC               # 720 logit columns
ZCH = 36                  # g-chunk size for the bl-update pipeline
NZCH = G // ZCH           # 2 chunks
XCH = 18                  # g-chunk size for xblk DMA (4 chunks/bg)
NXCH = G // XCH

_BUILT = None


def _consts():
    """Host-side constant tensors shared by all cores."""
    p = np.arange(128)
    bb_of_p = p % 8  # b-lane of partition (i_sub,b)

    # mcb [128, 80] bf16: delta(b(p) == b') at column (b'*10+o)
    col_b = (np.arange(80) // 10)
    mcb = (bb_of_p[:, None] == col_b[None, :]).astype(np.float32)

    # msks [80, 160] bf16: delta(o == o') ; row (b,o), col (o'*16+d)
    row_o = np.arange(80) % 10
    col_o = np.arange(160) // 16
    msks = (row_o[:, None] == col_o[None, :]).astype(np.float32)

    # arep [80, 128] bf16: delta(b == b') ; row (b,o), col (i_sub*8+b')
    row_b = np.arange(80) // 10
    col_b2 = np.arange(128) % 8
    arep = (row_b[:, None] == col_b2[None, :]).astype(np.float32)

    # sel [32, 512] bf16: sel[b, bg*128 + i_sub*8 + b8] = (b == bg*8+b8)
    sel = np.zeros((32, 4, 16, 8), np.float32)
    for bg in range(NBG):
        for b8 in range(BG):
            sel[bg * BG + b8, bg, :, b8] = 1.0
    sel = sel.reshape(32, 512)

    return {
        "mcb": mcb.astype(BF16),
        "mcf": mcb,  # f32 copy for per-partition scalar masks
        "msks": msks.astype(BF16),
        "arep": arep.astype(BF16),
        "sel": sel.astype(BF16),
    }


def _prep_core(x_c):
    """Per-core input prep. x_c: [32, 1152, 8] f32.
    xblk [NBG, 128, G*128] bf16 block-diagonal:
      xblk[bg, i_sub*8+k, g*128 + i_sub*8+b] = x_c[bg*8+b, g*16+i_sub, k]
    x2d [128, G*32] bf16 dense: x2d[i_sub*8+k, g*32+b] = x_c[b, g*16+i_sub, k]
    """
    xblk = np.zeros((NBG, 128, G * 128), np.float32)
    xv = x_c.reshape(NBG, BG, G, 16, KD)  # [bg, b, g, i_sub, k]
    for i_sub in range(16):
        blk = xv[:, :, :, i_sub, :].transpose(0, 3, 2, 1)  # [bg, k, g, b]
        xblk[:, i_sub * 8 : i_sub * 8 + 8, :].reshape(NBG, 8, G, 128)[
            :, :, :, i_sub * 8 : i_sub * 8 + 8
        ] = blk
    x2 = x_c.reshape(32, G, 16, KD).transpose(2, 3, 1, 0)  # [i_sub, k, g, b]
    x2d = np.ascontiguousarray(x2.reshape(128, G * 32))
    return {"xblk": xblk.astype(BF16), "x2d": x2d.astype(BF16)}


def _prep_w(W):
    """wr [128, G*160] bf16: wr[i_sub*8+k, g*160 + o*16+d] = W[g*16+i_sub,o,d,k]"""
    wv = W.reshape(G, 16, OC, OD, KD)  # [g, i_sub, o, d, k]
    wr = wv.transpose(1, 4, 0, 2, 3).reshape(128, G * ODF)
    return np.ascontiguousarray(wr).astype(BF16)


def _in_maps(x, W):
    x = np.asarray(x, np.float32)
    W = np.asarray(W, np.float32)
    wr = _prep_w(W)
    cst = _consts()
    in_maps = []
    for c in range(NCORES):
        m = _prep_core(x[c * BL : (c + 1) * BL])
        m["wr"] = wr
        m.update(cst)
        in_maps.append(m)
    return in_maps


def _build():
    global _BUILT
    if _BUILT is not None:
        return _BUILT

    nc = bacc_mod.Bacc()
    dt = mybir.dt
    xblk_d = nc.dram_tensor("xblk", [NBG, 128, G * 128], dt.bfloat16, kind="ExternalInput")
    x2d_d = nc.dram_tensor("x2d", [128, G * 32], dt.bfloat16, kind="ExternalInput")
    wr_d = nc.dram_tensor("wr", [128, G * ODF], dt.bfloat16, kind="ExternalInput")
    mcb_d = nc.dram_tensor("mcb", [128, 80], dt.bfloat16, kind="ExternalInput")
    mcf_d = nc.dram_tensor("mcf", [128, 80], dt.float32, kind="ExternalInput")
    msks_d = nc.dram_tensor("msks", [80, ODF], dt.bfloat16, kind="ExternalInput")
    arep_d = nc.dram_tensor("arep", [80, 128], dt.bfloat16, kind="ExternalInput")
    sel_d = nc.dram_tensor("sel", [32, 512], dt.bfloat16, kind="ExternalInput")
    vout_d = nc.dram_tensor("vout", [BL, OC, OD], dt.float32, kind="ExternalOutput")

    AF = mybir.ActivationFunctionType
    ALU = mybir.AluOpType
    AX = mybir.AxisListType

    with TileContext(nc) as tc:
        with (
            tc.tile_pool(name="consts", bufs=1) as cpool,
            tc.tile_pool(name="wrp", bufs=1) as wpool,
            tc.tile_pool(name="xbp", bufs=3) as xpool,
            tc.tile_pool(name="uhp", bufs=1) as uhpool,
            tc.tile_pool(name="blp", bufs=1) as blpool,
            tc.tile_pool(name="route", bufs=2) as rpool,
            tc.tile_pool(name="eep", bufs=1) as eepool,
            tc.tile_pool(name="ztmp", bufs=1) as zpool,
            tc.tile_pool(name="small", bufs=2) as spool,
            tc.tile_pool(name="vr", bufs=3) as vpool,
            tc.tile_pool(name="pe", bufs=2, space="PSUM") as pe_pool,
            tc.tile_pool(name="ps", bufs=2, space="PSUM") as ps_pool,
            tc.tile_pool(name="pv", bufs=1, space="PSUM") as pv_pool,
            tc.tile_pool(name="p0", bufs=1, space="PSUM") as p0_pool,
        ):
            # ---- resident constants / weights
            # wr in 6 per-slice tiles so matmuls only wait for their slice
            WSL = G // 6  # 12 groups per slice
            wr_t = []
            x2d = wpool.tile([128, G * 32], dt.bfloat16, tag="x2d")
            # split halves: s0's first matmuls only wait on the first half
            XH = G * 32 // 2
            nc.sync.dma_start(out=x2d[:, 0:XH], in_=x2d_d[:, 0:XH])
            nc.sync.dma_start(out=x2d[:, XH:], in_=x2d_d[:, XH:])
            for s in range(6):
                w = WSL * ODF
                wt = wpool.tile([128, w], dt.bfloat16, tag=f"wr{s}")
                nc.sync.dma_start(out=wt[:], in_=wr_d[:, s * w : (s + 1) * w])
                wr_t.append(wt)

            def wr_g(g):
                return wr_t[g // WSL][:, (g % WSL) * ODF : (g % WSL + 1) * ODF]
            mcb = cpool.tile([128, 80], dt.bfloat16, tag="mcb")
            nc.sync.dma_start(out=mcb[:], in_=mcb_d[:])
            mcf = cpool.tile([128, 80], dt.float32, tag="mcf")
            nc.sync.dma_start(out=mcf[:], in_=mcf_d[:])
            msks = cpool.tile([80, ODF], dt.bfloat16, tag="msks")
            nc.sync.dma_start(out=msks[:], in_=msks_d[:])
            arep = cpool.tile([80, 128], dt.bfloat16, tag="arep")
            nc.sync.dma_start(out=arep[:], in_=arep_d[:])
            sel = cpool.tile([32, 512], dt.bfloat16, tag="sel")
            nc.sync.dma_start(out=sel[:], in_=sel_d[:])
            czero = cpool.tile([128, 1], dt.float32, tag="czero")
            nc.vector.memset(czero[:], 0.0)
            ceps = cpool.tile([80, 1], dt.float32, tag="ceps")
            nc.vector.memset(ceps[:], 1e-8)



            vrep0_t = []

            def s0_phase():
                # s0 = 0.1 * sum_i u_hat  (dense accumulation, all 32 b)
                ps0 = p0_pool.tile([32, ODF], dt.float32, tag="ps0")
                for g in range(G):
                    nc.tensor.matmul(
                        ps0[:],
                        x2d[:, g * 32 : (g + 1) * 32],
                        wr_g(g),
                        start=(g == 0),
                        stop=(g == G - 1),
                    )
                # squash on [32, ...]: v0 = fac*s0, fac = ns/((1+ns)sqrt(ns+eps)),
                # s0 = 0.1*T (T = ps0); ns from Square(0.1*T).
                sq0 = spool.tile([32, ODF], dt.float32, tag="sq0")
                nc.scalar.activation(
                    out=sq0[:], in_=ps0[:], func=AF.Square, bias=czero[:32], scale=0.1
                )
                ns0 = spool.tile([32, OC], dt.float32, tag="ns0")
                nc.vector.tensor_reduce(
                    out=ns0[:],
                    in_=sq0[:].rearrange("p (o d) -> p o d", o=OC),
                    axis=AX.X,
                    op=ALU.add,
                )

                # sqrt via exp(0.5*ln): keeps every Act func in ONE table set
                # (natural_log_exp_and_others) - Sqrt shares no set with Exp,
                # so using it forced 1.3us ACT_TABLE_LOADs at every seam.
                lns0 = spool.tile([32, OC], dt.float32, tag="lns0")
                nc.scalar.activation(
                    out=lns0[:], in_=ns0[:], func=AF.Ln, bias=ceps[:32]
                )
                sqn0 = spool.tile([32, OC], dt.float32, tag="sqn0")
                nc.scalar.activation(
                    out=sqn0[:], in_=lns0[:], func=AF.Exp, bias=czero[:32], scale=0.5
                )
                den0 = spool.tile([32, OC], dt.float32, tag="den0")
                nc.vector.scalar_tensor_tensor(
                    out=den0[:], in0=ns0[:], scalar=1.0, in1=sqn0[:],
                    op0=ALU.add, op1=ALU.mult,
                )
                rden0 = spool.tile([32, OC], dt.float32, tag="rden0")
                nc.vector.reciprocal(out=rden0[:], in_=den0[:])
                fac0 = spool.tile([32, OC], dt.float32, tag="fac0")
                # fac = 0.1 * ns * rden  (0.1 for s0 = 0.1*T)
                nc.vector.tensor_tensor(
                    out=fac0[:], in0=ns0[:], in1=rden0[:], op=ALU.mult
                )
                nc.vector.tensor_scalar_mul(fac0[:], fac0[:], 0.1)
                v0_bf = spool.tile([32, ODF], dt.bfloat16, tag="v0_bf")
                nc.vector.tensor_tensor(
                    out=v0_bf[:].rearrange("p (o d) -> p o d", o=OC),
                    in0=ps0[:].rearrange("p (o d) -> p o d", o=OC),
                    in1=fac0[:].unsqueeze(2).broadcast_to([32, OC, OD]),
                    op=ALU.mult,
                )
                return v0_bf

            def s0_vreps(v0_bf):
                # vrep0 per bgroup via select matmul. Emitted AFTER some
                # einsum chunks: queueing these on PE right after the s0
                # matmuls stalled PE ~5us waiting on the squash chain.
                for bg in range(NBG):
                    pv = pv_pool.tile([128, ODF], dt.float32, tag="pv")
                    nc.tensor.matmul(
                        pv[:], sel[:, bg * 128 : (bg + 1) * 128], v0_bf[:],
                        start=True, stop=True,
                    )
                    vr = vpool.tile([128, ODF], dt.bfloat16, tag=f"vrep0{bg}")
                    nc.scalar.copy(out=vr[:], in_=pv[:])
                    vrep0_t.append(vr)

            # ---- Phase A: einsum for all bgroups
            uh_t = []
            bl_t = []
            for bg in range(NBG):
                uh = uhpool.tile([128, G * ODF], dt.bfloat16, tag=f"uh{bg}")
                uh_t.append(uh)
                bl = blpool.tile([128, GO], dt.bfloat16, tag=f"bl{bg}")
                bl_t.append(bl)

            def einsum_bg(bg, xcs=None):
                uh = uh_t[bg]
                for xc in xcs if xcs is not None else range(NXCH):
                    xt = xpool.tile([128, XCH * 128], dt.bfloat16, tag="xt")
                    nc.sync.dma_start(
                        out=xt[:],
                        in_=xblk_d[bg][:, xc * XCH * 128 : (xc + 1) * XCH * 128],
                    )
                    for t in range(XCH // 6):
                        pe = pe_pool.tile([128, 960], dt.float32, tag="pe")
                        for j in range(6):
                            gl = t * 6 + j           # local g in chunk
                            g = xc * XCH + gl        # global g
                            nc.tensor.matmul(
                                pe[:, j * ODF : (j + 1) * ODF],
                                xt[:, gl * 128 : (gl + 1) * 128],
                                wr_g(g),
                                start=True,
                                stop=True,
                            )
                        g0 = xc * XCH + t * 6
                        if bg == 0 and t % 2 == 1:
                            nc.vector.tensor_copy(
                                out=uh[:, g0 * ODF : (g0 + 6) * ODF], in_=pe[:]
                            )
                        else:
                            nc.scalar.copy(
                                out=uh[:, g0 * ODF : (g0 + 6) * ODF], in_=pe[:]
                            )

            GPS_CHUNK = False  # GpSimd bl-update chunk: measured far slower

            def bl_update(bg, vrep, first):
                """bl[bg] (+)= sum_d uh[bg]*vrep ; first=True writes fresh.
                Last chunk runs entirely on GpSimd in parallel with VectorE
                doing the other three."""
                bl = bl_t[bg]
                for ch in range(NZCH):
                    eng = nc.gpsimd if (GPS_CHUNK and ch == NZCH - 1) else nc.vector
                    tail = eng
                    cs = ch * ZCH
                    z = zpool.tile([128, ZCH * ODF], dt.bfloat16,
                                   tag="z")
                    eng.tensor_tensor(
                        out=z[:].rearrange("p (g f) -> p g f", f=ODF),
                        in0=uh_t[bg][:, cs * ODF : (cs + ZCH) * ODF].rearrange(
                            "p (g f) -> p g f", f=ODF
                        ),
                        in1=vrep[:].unsqueeze(1).broadcast_to([128, ZCH, ODF]),
                        op=ALU.mult,
                    )
                    t8 = zpool.tile([128, ZCH * 80], dt.bfloat16,
                                    tag="t8")
                    zv = z[:].rearrange("p (g o d) -> p g o d", o=OC, d=OD)
                    eng.tensor_tensor(
                        out=t8[:].rearrange("p (g o d) -> p g o d", o=OC, d=8),
                        in0=zv[:, :, :, 0:8],
                        in1=zv[:, :, :, 8:16],
                        op=ALU.add,
                    )
                    t4 = zpool.tile([128, ZCH * 40], dt.bfloat16,
                                    tag="t4")
                    t8v = t8[:].rearrange("p (g o d) -> p g o d", o=OC, d=8)
                    eng.tensor_tensor(
                        out=t4[:].rearrange("p (g o d) -> p g o d", o=OC, d=4),
                        in0=t8v[:, :, :, 0:4],
                        in1=t8v[:, :, :, 4:8],
                        op=ALU.add,
                    )
                    t2 = zpool.tile([128, ZCH * 20], dt.bfloat16,
                                    tag="t2")
                    t4v = t4[:].rearrange("p (g o d) -> p g o d", o=OC, d=4)
                    tail.tensor_tensor(
                        out=t2[:].rearrange("p (g o d) -> p g o d", o=OC, d=2),
                        in0=t4v[:, :, :, 0:2],
                        in1=t4v[:, :, :, 2:4],
                        op=ALU.add,
                    )
                    t2v = t2[:].rearrange("p (g o d) -> p g o d", o=OC, d=2)
                    blv = bl[:, cs * OC : (cs + ZCH) * OC]
                    if first:
                        tail.tensor_tensor(
                            out=blv.rearrange("p (g o) -> p g o", o=OC).unsqueeze(3),
                            in0=t2v[:, :, :, 0:1],
                            in1=t2v[:, :, :, 1:2],
                            op=ALU.add,
                        )
                    else:
                        t1 = zpool.tile([128, ZCH * OC], dt.bfloat16,
                                        tag="t1")
                        tail.tensor_tensor(
                            out=t1[:].rearrange("p (g o) -> p g o", o=OC).unsqueeze(3),
                            in0=t2v[:, :, :, 0:1],
                            in1=t2v[:, :, :, 1:2],
                            op=ALU.add,
                        )
                        tail.tensor_tensor(
                            out=blv, in0=blv, in1=t1[:], op=ALU.add
                        )

            # ---- iter-0: einsum interleaved with bl updates (software pipeline)
            # s0 FIRST: its dense matmul needs only x2d+wr (no xblk), so v0/
            # vrep0 are ready ~5us earlier and the DVE update chain starts
            # as soon as uh(0) lands.
            v0_bf = s0_phase()
            einsum_bg(0, xcs=(0, 1))
            s0_vreps(v0_bf)
            einsum_bg(0, xcs=(2, 3))
            einsum_bg(1)
            bl_update(0, vrep0_t[0], first=True)
            einsum_bg(2)
            bl_update(1, vrep0_t[1], first=True)
            einsum_bg(3)

            # ---- iters 1,2: stages, software-pipelined across (it,bg) columns
            col_state = {}

            # persistent ee per bg: it2 uses ee2 = exp(delta)*ee1 so the it1
            # bl-update can write its delta via the cheap first=True path
            # (no t1 + bl-add TTs on VectorE)
            ee1_t = [None] * NBG

            def stage1(it, bg, vcall_all=False):
                """softmax over o + call build. vcall_all: build every b'
                lane on VectorE (last column: keeps ScalarE off the exposed
                tail chain)."""
                bl = bl_t[bg]
                if it == 1:
                    ee = eepool.tile([128, GO], dt.bfloat16, tag=f"ee1_{bg}")
                    ee1_t[bg] = ee
                    nc.scalar.activation(
                        out=ee[:], in_=bl[:], func=AF.Exp, bias=czero[:]
                    )
                else:
                    eeD = rpool.tile([128, GO], dt.bfloat16, tag="eeD")
                    nc.scalar.activation(
                        out=eeD[:], in_=bl[:], func=AF.Exp, bias=czero[:]
                    )
                    ee = rpool.tile([128, GO], dt.bfloat16, tag="ee")
                    nc.vector.tensor_tensor(
                        out=ee[:], in0=eeD[:], in1=ee1_t[bg][:], op=ALU.mult
                    )
                zz = rpool.tile([128, G], dt.float32, tag="zz")
                nc.vector.tensor_reduce(
                    out=zz[:],
                    in_=ee[:].rearrange("p (g o) -> p g o", o=OC),
                    axis=AX.X,
                    op=ALU.add,
                )
                rz = rpool.tile([128, G], dt.float32, tag="rz")
                nc.vector.reciprocal_approx_fast(out=rz[:], in_=zz[:])
                # cCb = ee * (1/Z) in ONE TT with broadcast in1 (runs 1x, but
                # replaces the old rzb CAST@1x + cCb TT@2x pair)
                cCb = rpool.tile([128, GO], dt.bfloat16, tag="cCb")
                nc.vector.tensor_tensor(
                    out=cCb[:].rearrange("p (g o) -> p g o", o=OC),
                    in0=ee[:].rearrange("p (g o) -> p g o", o=OC),
                    in1=rz[:].unsqueeze(2).broadcast_to([128, G, OC]),
                    op=ALU.mult,
                )
                call = rpool.tile([128, G * 80], dt.bfloat16, tag="call")
                callv = call[:].rearrange(
                    "p (g b o) -> p g b o", b=BG, o=OC
                )
                # call build: split 2 lanes VectorE tensor_scalar / 6 ScalarE
                # masked copies (all-Act delays the s-matmul chain; all-DVE
                # loads the binding engine). vcall_all (last column): all on
                # VectorE so no ScalarE hop sits on the exposed tail chain.
                nv = BG if vcall_all else 2
                cCv = cCb[:].rearrange("p (g o) -> p g o", o=OC)
                for bp in range(nv):
                    nc.vector.tensor_scalar_mul(
                        callv[:, :, bp, :], cCv, mcf[:, bp * OC : bp * OC + 1]
                    )
                for bp in range(nv, BG):
                    nc.scalar.activation(
                        out=callv[:, :, bp, :],
                        in_=cCb[:].rearrange("p (g o) -> p g o", o=OC),
                        func=AF.Copy,
                        bias=0.0,
                        scale=mcf[:, bp * OC : bp * OC + 1],
                    )
                col_state[(it, bg)] = {"call": call}

            def stage2(it, bg, split=False):
                """s matmul: accumulate over all 72 groups."""
                call = col_state[(it, bg)]["call"]
                ps = ps_pool.tile([80, ODF], dt.float32, tag="ps")
                for g in range(G):
                    nc.tensor.matmul(
                        ps[:],
                        call[:, g * 80 : (g + 1) * 80],
                        uh_t[bg][:, g * ODF : (g + 1) * ODF],
                        start=(g == 0),
                        stop=(g == G - 1),
                    )
                col_state[(it, bg)]["ps"] = ps

            def stage3(it, bg, last=False):
                """diag extract + squash; it1: build vrep + bl update input,
                it2: final v -> DRAM. last: the tail column - keep the chain
                off ScalarE where DVE is idle anyway."""
                # extract diag o==o' -> s_t [80,16]
                tmp = spool.tile([80, ODF], dt.float32, tag="tmp")
                mskv = (
                    msks[:]
                    .rearrange("p (o d) -> p o d", o=OC)
                    .transpose([0, 2, 1])
                )
                if "ps2" in col_state[(it, bg)]:
                    ps2 = col_state[(it, bg)]["ps2"]
                    tmh = col_state[(it, bg)]["tmh"]
                    tm2 = cpool.tile([80, ODF], dt.float32, tag="tm2")
                    nc.vector.tensor_tensor(
                        out=tm2[:].rearrange("p (d o) -> p d o", o=OC),
                        in0=ps2[:, ODF : 2 * ODF]
                        .rearrange("p (o d) -> p o d", o=OC)
                        .transpose([0, 2, 1]),
                        in1=mskv,
                        op=ALU.mult,
                    )
                    nc.vector.tensor_tensor(
                        out=tmp[:], in0=tmh[:], in1=tm2[:], op=ALU.add
                    )
                else:
                    ps = col_state[(it, bg)]["ps"]
                    nc.vector.tensor_tensor(
                        out=tmp[:].rearrange("p (d o) -> p d o", o=OC),
                        in0=ps[:]
                        .rearrange("p (o d) -> p o d", o=OC)
                        .transpose([0, 2, 1]),
                        in1=mskv,
                        op=ALU.mult,
                    )
                s_t = spool.tile([80, OD], dt.float32, tag="s_t")
                nc.vector.tensor_reduce(
                    out=s_t[:],
                    in_=tmp[:].rearrange("p (d o) -> p d o", o=OC),
                    axis=AX.X,
                    op=ALU.add,
                )

                # squash: fac = ns / ((1+ns) * sqrt(ns+eps))
                # (tried Square on DVE for the last column: measured ~0.8us
                # WORSE - keep it on ScalarE)
                sq = spool.tile([80, OD], dt.float32, tag="sq")
                ns = spool.tile([80, 1], dt.float32, tag="ns")
                nc.scalar.activation(
                    out=sq[:], in_=s_t[:], func=AF.Square, bias=czero[:80]
                )
                nc.vector.tensor_reduce(
                    out=ns[:], in_=sq[:], axis=AX.X, op=ALU.add
                )
                lns = spool.tile([80, 1], dt.float32, tag="lns")
                nc.scalar.activation(
                    out=lns[:], in_=ns[:], func=AF.Ln, bias=ceps[:]
                )
                sqn = spool.tile([80, 1], dt.float32, tag="sqn")
                nc.scalar.activation(
                    out=sqn[:], in_=lns[:], func=AF.Exp, bias=czero[:80], scale=0.5
                )
                den = spool.tile([80, 1], dt.float32, tag="den")
                nc.vector.scalar_tensor_tensor(
                    out=den[:], in0=ns[:], scalar=1.0, in1=sqn[:],
                    op0=ALU.add, op1=ALU.mult,
                )
                rden = spool.tile([80, 1], dt.float32, tag="rden")
                nc.vector.reciprocal(out=rden[:], in_=den[:])
                fac = spool.tile([80, 1], dt.float32, tag="fac")
                nc.vector.tensor_tensor(
                    out=fac[:], in0=ns[:], in1=rden[:], op=ALU.mult
                )

                if it == ITERS - 1:
                    v_f = spool.tile([80, OD], dt.float32, tag="v_f")
                    nc.vector.tensor_scalar_mul(v_f[:], s_t[:], fac[:])
                    nc.sync.dma_start(
                        out=vout_d[bg * BG : (bg + 1) * BG].rearrange(
                            "b o d -> (b o) d"
                        ),
                        in_=v_f[:],
                    )
                    return

                v_bf = spool.tile([80, OD], dt.bfloat16, tag="v_bf")
                nc.vector.tensor_scalar_mul(v_bf[:], s_t[:], fac[:])

                # vexp[(b,o),(o',d)] = v[b,o,d] * delta(o==o')
                vexp = spool.tile([80, ODF], dt.bfloat16, tag="vexp")
                nc.vector.tensor_tensor(
                    out=vexp[:].rearrange("p (o d) -> p o d", o=OC),
                    in0=msks[:].rearrange("p (o d) -> p o d", o=OC),
                    in1=v_bf[:].unsqueeze(1).broadcast_to([80, OC, OD]),
                    op=ALU.mult,
                )
                pv = pv_pool.tile([128, ODF], dt.float32, tag="pv")
                nc.tensor.matmul(pv[:], arep[:], vexp[:], start=True, stop=True)
                vrep = vpool.tile([128, ODF], dt.bfloat16, tag="vrep")
                nc.scalar.copy(out=vrep[:], in_=pv[:])
                col_state[(it, bg)]["vrep"] = vrep

            # pipelined emission: stage1(n) | stage3(n-1) | stage2(n) |
            # bl_update(n-1, it1 only)
            # seam: start the it1 pipeline for bg0 between iter-0 updates so
            # ScalarE's exp is queued before the last iter-0 bl-updates
            stage1(1, 0)
            bl_update(2, vrep0_t[2], first=True)
            stage2(1, 0)
            bl_update(3, vrep0_t[3], first=True)

            # it2 ends (2,3),(2,2): the final column's bl (bg2) is updated a
            # slot earlier, and the last two s-matmuls run back-to-back on a
            # warm PE while stage3(2,3) overlaps s-matmul(2,2).
            cols = [(1, 0), (1, 1), (1, 2), (1, 3), (2, 0), (2, 1), (2, 3), (2, 2)]
            # bl-update lags TWO columns so VectorE never waits on the
            # in-flight s-matmul; bg2's update stays last so VectorE
            # stays busy during the final it2 s-matmuls
            upd_at = {2: 0, 3: 1, 5: 3, 6: 2}
            for n in range(1, len(cols)):
                stage1(*cols[n], vcall_all=(n == len(cols) - 1))
                if n in upd_at:
                    c = cols[upd_at[n]]
                    # first=True: write the it1 delta over bl (dead after
                    # ee1 = exp(bl) was taken); it2 recombines via ee1*exp(d)
                    bl_update(c[1], col_state[c]["vrep"], first=True)
                stage3(*cols[n - 1])
                stage2(*cols[n])
            stage3(*cols[-1], last=True)

    nc.finalize()
    _BUILT = nc
    return nc


_WARMED = False


def kernel(x, W):
    global _WARMED
    nc = _build()
    in_maps = _in_maps(x, W)
    if not _WARMED:
        # First execution after an in-process compile can return a
        # partially-unwritten output buffer (observed: bgroup 0 rows NaN).
        # Run once with the real inputs and discard.
        run_bass_kernel_spmd(nc, in_maps, core_ids=list(range(NCORES)))
        _WARMED = True
    res = run_bass_kernel_spmd(nc, in_maps, core_ids=list(range(NCORES)))
    outs = res.results
    v = np.concatenate([np.asarray(o["vout"]) for o in outs], axis=0)
    return v.astype(np.float32)


if __name__ == "__main__":
    rng = np.random.default_rng(0)
    x = rng.standard_normal((B, IC, KD), np.float32)
    W = rng.standard_normal((IC, OC, OD, KD), np.float32)
    v = kernel(x, W)
    print("out", v.shape, v.dtype, float(np.abs(v).mean()))



# revision 43
# speedup vs baseline: 1.0057x; 1.0057x over previous
"""DigitCaps (CapsNet dynamic routing) Trainium2 kernel, v3.

v3 changes (193us -> ~185us, rel err 3.9e-3; VectorE busy 152 -> ~146us):
  - act-table pin: Sqrt -> exp(0.5*ln(x)) and a monkeypatch restricting
    Exp/Ln/Square/Copy/Identity to the single set
    natural_log_exp_and_others. ACT_TABLE_LOADs: 10x1.3us -> 1.
    (Plain Ln+Exp WITHOUT the pin made it WORSE - 19 loads - because the
    pass maps Ln to natural_log which lacks Exp.)
  - rzb CAST folded into cCb: one TT with broadcast rz (1x) replaces
    rzb-copy(1x)+cCb(2x): -0.5us/col.
  - bl and msks in bf16 (tree writes/accumulates at 2x).
  - exp-product: it1 bl-updates write their delta via the cheap
    first=True path; it2 uses ee2 = exp(delta)*ee1 (persistent ee1 per
    bg in eepool). Drops t1+bl-add TTs.
  - call build 2 lanes VectorE tensor_scalar + 6 ScalarE masked copies
    (all-Act delays s-matmuls +3us; all-DVE loads the binding engine).
  - s0 BEFORE einsum (needs only x2d+wr); vrep0 select-matmuls emitted
    between einsum xc-chunks (emitting them right after s0 stalled PE
    ~5us waiting on the squash chain). x2d DMA split in halves.
  MEASURED DEAD ENDS v3: PE warm-up dummy matmuls (p-state model wrong on
  HW - real gate is ~50% power throttle, util limit 0.7 avg); DVE TT/STT
  reading PSUM twice (walrus verifier rejects); gpsimd.tensor_copy from
  PSUM (verifier rejects); ps_pool bufs=3 (PSUM banks exhausted);
  tensor_scalar 4x mode (cost model says 4x_2p, HW runs 2x: 363ns for
  720 elems); TensorReduce has NO fast modes (1x) - the TT pairwise tree
  beats it everywhere.
  Remaining wall (185us): ~22 startup (launch 7 + s0+squash chain ~14,
  einsum pace ~94ns/matmul-slot under throttle), ~146 VectorE-dense
  (z 50 + t8 26 + t4 14 + t2 9 + softmax/call ~20 + stage3 ~8 + iter0
  copies/casts ~5 + misc), ~12 scattered sub-us sem-latency gaps, ~5
  tail (last s-matmul + extract + out-DMA flush).

Previous docstring (v2):

Math (per reference):
  u_hat[b,i,o,d] = sum_k W[i,o,d,k] * x[b,i,k]      B=256, IC=1152, K=8, O=10, D=16
  3 routing iters: c = softmax_o(bl); s = sum_i c*u_hat; v = squash(s);
                   bl += sum_d u_hat*v
  out v: [B, 10, 16]

Data-parallel over batch: 8 cores x 32 samples, 4 bgroups of 8 per core.
Einsum on TensorE with block-diagonal x (lhsT stationary, wr moving).
v2 design (326us baseline -> 198us, rel err 1.8e-2 -> 3.9e-3):
  - iter-0 s is a DENSE matmul: c uniform -> s0 = 0.1*sum_i u_hat
    = accumulation of x2d[g].T @ wr[g] over all 72 groups (out [32,160],
    f32 PSUM accumulate - also the big accuracy win). Removes 288
    routing matmuls + 4 diag extracts; v0 broadcast to (i16,b8) rows
    via a tiny select matmul per bgroup.
  - bl-update (the VectorE-bound core: ~15us per (iter,bgroup)):
    z = uh*vrep (bf16 2x mode) then pairwise d-tree, 2 chunks of 36
    groups; last tree level writes/accumulates bl directly.
  - softmax: exp->bf16, reciprocal_approx_fast for 1/Z, cC mult
    outputs bf16 directly (cast folded).
  - call build split: b' 0:4 on VectorE (one broadcast TT), b' 4:8 on
    ScalarE (per-partition-scalar masked copies) - keeps the critical
    VectorE queue short. The LAST column builds all 8 b' on VectorE
    (vcall_all) so no ScalarE hop sits on the exposed tail chain
    (trace-verified: tail idle 9.4us -> 5.9us).
  - software-pipelined emission across (iter,bgroup) columns: stage1
    (softmax+call), stage2 (s-matmul, TensorE), stage3 (extract+squash
    +vrep), bl-update lagged per upd_at (lag 2; last one lag 3 to cover
    the final it2 s-matmuls); first it1 column interleaved into iter-0;
    bg0's einsum copies alternate ScalarE/VectorE (V idle at start).
  - wr in 6 per-slice tiles so early matmuls don't wait the full DMA.
  Wall accounting (195us median, 193-198 device band): ~162us VectorE
  at the 2-elem/cyc bf16 ceiling (the bl-update d-contraction cannot
  leave DVE: PE contracts over partitions only - every reformulation
  hits a b-pairing or o-diag-extraction wall; ScalarE has no 2-tensor
  op), ~20us startup (8us launch + 5.9MB critical set at ~305GB/s DMA
  roofline), ~6us tail (exp + final s-matmul; split-accumulation for
  early extract measured worse twice).
  MEASURED DEAD ENDS - do not retry without new hardware facts:
  GpSimd tensor_tensor (4-8x slower than DVE, poisons pipeline; tried
  call halves, tree tails, full bl-update chunks); gpsimd.dma_start
  (routes to slow software-DGE); fp8 s-matmul (s ~= sum of cancelling
  terms so quant error hits s at full 3.6%, not /sqrt(N)); multi-dim
  strided stationary AP for LDWEIGHTS (compiler rejects); d-major t2
  tree level (strided write penalty); uh half/quarter tiles; s0-first
  T-queue order; deeper bl-update lag shuffles.
"""

import sys

sys.path.insert(0, "/opt/trn_rl_repo")

import numpy as np
import ml_dtypes

import concourse.bass as bass
import concourse.bacc as bacc_mod
from concourse import mybir
from concourse.tile import TileContext
from concourse.bass_utils import run_bass_kernel_spmd

# Pin every Act func we use (Exp/Ln/Square/Copy/Identity) to the single
# covering table set `natural_log_exp_and_others` so the act-table pass
# can never alternate sets (each ACT_TABLE_LOAD costs 1.3us on ScalarE;
# baseline thrashes 10-19 of them). Set keys/order are preserved so the
# act_func_set_id -> act_info.json index mapping stays valid.
_ONE_SET = "natural_log_exp_and_others"


def _pin_act_tables():
    import concourse.hw_specs as hw_specs

    orig = hw_specs.get_activation_tables
    if getattr(orig, "_digitcaps_pinned", False):
        return
    _pin = {"Exp", "Ln", "Square", "Copy", "Identity"}

    def patched(module_arch):
        t = orig(module_arch)
        out = {}
        for k, v in t.items():
            if k == _ONE_SET:
                out[k] = v
            else:
                out[k] = {f for f in v if f.name not in _pin}
        return out

    patched._digitcaps_pinned = True
    hw_specs.get_activation_tables = patched
    bacc_mod.get_activation_tables = patched


_pin_act_tables()

BF16 = ml_dtypes.bfloat16

# Problem dims (hardcoded per harness contract)
B, IC, KD, OC, OD = 256, 1152, 8, 10, 16
NCORES = 8
BL = B // NCORES          # 32 samples per core
BG = 8                    # bgroup size
NBG = BL // BG            # 4 bgroups
G = IC // 16              # 72 groups of 16 in-caps
ODF = OC * OD             # 160
ITERS = 3
GO = G * OC               # 720 logit columns
ZCH = 36                  # g-chunk size for the bl-update pipeline
NZCH = G // ZCH           # 2 chunks
XCH = 18                  # g-chunk size for xblk DMA (4 chunks/bg)
NXCH = G // XCH

_BUILT = None


def _consts():
    """Host-side constant tensors shared by all cores."""
    p = np.arange(128)
    bb_of_p = p % 8  # b-lane of partition (i_sub,b)

    # mcb [128, 80] bf16: delta(b(p) == b') at column (b'*10+o)
    col_b = (np.arange(80) // 10)
    mcb = (bb_of_p[:, None] == col_b[None, :]).astype(np.float32)

    # msks [80, 160] bf16: delta(o == o') ; row (b,o), col (o'*16+d)
    row_o = np.arange(80) % 10
    col_o = np.arange(160) // 16
    msks = (row_o[:, None] == col_o[None, :]).astype(np.float32)

    # arep [80, 128] bf16: delta(b == b') ; row (b,o), col (i_sub*8+b')
    row_b = np.arange(80) // 10
    col_b2 = np.arange(128) % 8
    arep = (row_b[:, None] == col_b2[None, :]).astype(np.float32)

    # sel [32, 512] bf16: sel[b, bg*128 + i_sub*8 + b8] = (b == bg*8+b8)
    sel = np.zeros((32, 4, 16, 8), np.float32)
    for bg in range(NBG):
        for b8 in range(BG):
            sel[bg * BG + b8, bg, :, b8] = 1.0
    sel = sel.reshape(32, 512)

    return {
        "mcb": mcb.astype(BF16),
        "mcf": mcb,  # f32 copy for per-partition scalar masks
        "msks": msks.astype(BF16),
        "arep": arep.astype(BF16),
        "sel": sel.astype(BF16),
    }


def _prep_core(x_c):
    """Per-core input prep. x_c: [32, 1152, 8] f32.
    xblk [NBG, 128, G*128] bf16 block-diagonal:
      xblk[bg, i_sub*8+k, g*128 + i_sub*8+b] = x_c[bg*8+b, g*16+i_sub, k]
    x2d [128, G*32] bf16 dense: x2d[i_sub*8+k, g*32+b] = x_c[b, g*16+i_sub, k]
    """
    xblk = np.zeros((NBG, 128, G * 128), np.float32)
    xv = x_c.reshape(NBG, BG, G, 16, KD)  # [bg, b, g, i_sub, k]
    for i_sub in range(16):
        blk = xv[:, :, :, i_sub, :].transpose(0, 3, 2, 1)  # [bg, k, g, b]
        xblk[:, i_sub * 8 : i_sub * 8 + 8, :].reshape(NBG, 8, G, 128)[
            :, :, :, i_sub * 8 : i_sub * 8 + 8
        ] = blk
    x2 = x_c.reshape(32, G, 16, KD).transpose(2, 3, 1, 0)  # [i_sub, k, g, b]
    x2d = np.ascontiguousarray(x2.reshape(128, G * 32))
    return {"xblk": xblk.astype(BF16), "x2d": x2d.astype(BF16)}


def _prep_w(W):
    """wr [128, G*160] bf16: wr[i_sub*8+k, g*160 + o*16+d] = W[g*16+i_sub,o,d,k]"""
    wv = W.reshape(G, 16, OC, OD, KD)  # [g, i_sub, o, d, k]
    wr = wv.transpose(1, 4, 0, 2, 3).reshape(128, G * ODF)
    return np.ascontiguousarray(wr).astype(BF16)


def _in_maps(x, W):
    x = np.asarray(x, np.float32)
    W = np.asarray(W, np.float32)
    wr = _prep_w(W)
    cst = _consts()
    in_maps = []
    for c in range(NCORES):
        m = _prep_core(x[c * BL : (c + 1) * BL])
        m["wr"] = wr
        m.update(cst)
        in_maps.append(m)
    return in_maps


def _build():
    global _BUILT
    if _BUILT is not None:
        return _BUILT

    nc = bacc_mod.Bacc()
    dt = mybir.dt
    xblk_d = nc.dram_tensor("xblk", [NBG, 128, G * 128], dt.bfloat16, kind="ExternalInput")
    x2d_d = nc.dram_tensor("x2d", [128, G * 32], dt.bfloat16, kind="ExternalInput")
    wr_d = nc.dram_tensor("wr", [128, G * ODF], dt.bfloat16, kind="ExternalInput")
    mcb_d = nc.dram_tensor("mcb", [128, 80], dt.bfloat16, kind="ExternalInput")
    mcf_d = nc.dram_tensor("mcf", [128, 80], dt.float32, kind="ExternalInput")
    msks_d = nc.dram_tensor("msks", [80, ODF], dt.bfloat16, kind="ExternalInput")
    arep_d = nc.dram_tensor("arep", [80, 128], dt.bfloat16, kind="ExternalInput")
    sel_d = nc.dram_tensor("sel", [32, 512], dt.bfloat16, kind="ExternalInput")
    vout_d = nc.dram_tensor("vout", [BL, OC, OD], dt.float32, kind="ExternalOutput")

    AF = mybir.ActivationFunctionType
    ALU = mybir.AluOpType
    AX = mybir.AxisListType

    with TileContext(nc) as tc:
        with (
            tc.tile_pool(name="consts", bufs=1) as cpool,
            tc.tile_pool(name="wrp", bufs=1) as wpool,
            tc.tile_pool(name="xbp", bufs=3) as xpool,
            tc.tile_pool(name="uhp", bufs=1) as uhpool,
            tc.tile_pool(name="blp", bufs=1) as blpool,
            tc.tile_pool(name="route", bufs=2) as rpool,
            tc.tile_pool(name="eep", bufs=1) as eepool,
            tc.tile_pool(name="ztmp", bufs=1) as zpool,
            tc.tile_pool(name="small", bufs=2) as spool,
            tc.tile_pool(name="vr", bufs=3) as vpool,
            tc.tile_pool(name="pe", bufs=2, space="PSUM") as pe_pool,
            tc.tile_pool(name="ps", bufs=2, space="PSUM") as ps_pool,
            tc.tile_pool(name="pv", bufs=1, space="PSUM") as pv_pool,
            tc.tile_pool(name="p0", bufs=1, space="PSUM") as p0_pool,
        ):
            # ---- resident constants / weights
            # wr in 6 per-slice tiles so matmuls only wait for their slice
            WSL = G // 6  # 12 groups per slice
            wr_t = []
            x2d = wpool.tile([128, G * 32], dt.bfloat16, tag="x2d")
            # split halves: s0's first matmuls only wait on the first half
            XH = G * 32 // 2
            nc.sync.dma_start(out=x2d[:, 0:XH], in_=x2d_d[:, 0:XH])
            nc.sync.dma_start(out=x2d[:, XH:], in_=x2d_d[:, XH:])
            for s in range(6):
                w = WSL * ODF
                wt = wpool.tile([128, w], dt.bfloat16, tag=f"wr{s}")
                nc.sync.dma_start(out=wt[:], in_=wr_d[:, s * w : (s + 1) * w])
                wr_t.append(wt)

            def wr_g(g):
                return wr_t[g // WSL][:, (g % WSL) * ODF : (g % WSL + 1) * ODF]
            mcb = cpool.tile([128, 80], dt.bfloat16, tag="mcb")
            nc.sync.dma_start(out=mcb[:], in_=mcb_d[:])
            mcf = cpool.tile([128, 80], dt.float32, tag="mcf")
            nc.sync.dma_start(out=mcf[:], in_=mcf_d[:])
            msks = cpool.tile([80, ODF], dt.bfloat16, tag="msks")
            nc.sync.dma_start(out=msks[:], in_=msks_d[:])
            arep = cpool.tile([80, 128], dt.bfloat16, tag="arep")
            nc.sync.dma_start(out=arep[:], in_=arep_d[:])
            sel = cpool.tile([32, 512], dt.bfloat16, tag="sel")
            nc.sync.dma_start(out=sel[:], in_=sel_d[:])
            czero = cpool.tile([128, 1], dt.float32, tag="czero")
            nc.vector.memset(czero[:], 0.0)
            ceps = cpool.tile([80, 1], dt.float32, tag="ceps")
            nc.vector.memset(ceps[:], 1e-8)



            vrep0_t = []

            def s0_phase():
                # s0 = 0.1 * sum_i u_hat  (dense accumulation, all 32 b)
                ps0 = p0_pool.tile([32, ODF], dt.float32, tag="ps0")
                for g in range(G):
                    nc.tensor.matmul(
                        ps0[:],
                        x2d[:, g * 32 : (g + 1) * 32],
                        wr_g(g),
                        start=(g == 0),
                        stop=(g == G - 1),
                    )
                # squash on [32, ...]: v0 = fac*s0, fac = ns/((1+ns)sqrt(ns+eps)),
                # s0 = 0.1*T (T = ps0); ns from Square(0.1*T).
                sq0 = spool.tile([32, ODF], dt.float32, tag="sq0")
                nc.scalar.activation(
                    out=sq0[:], in_=ps0[:], func=AF.Square, bias=czero[:32], scale=0.1
                )
                ns0 = spool.tile([32, OC], dt.float32, tag="ns0")
                nc.vector.tensor_reduce(
                    out=ns0[:],
                    in_=sq0[:].rearrange("p (o d) -> p o d", o=OC),
                    axis=AX.X,
                    op=ALU.add,
                )

                # sqrt via exp(0.5*ln): keeps every Act func in ONE table set
                # (natural_log_exp_and_others) - Sqrt shares no set with Exp,
                # so using it forced 1.3us ACT_TABLE_LOADs at every seam.
                lns0 = spool.tile([32, OC], dt.float32, tag="lns0")
                nc.scalar.activation(
                    out=lns0[:], in_=ns0[:], func=AF.Ln, bias=ceps[:32]
                )
                sqn0 = spool.tile([32, OC], dt.float32, tag="sqn0")
                nc.scalar.activation(
                    out=sqn0[:], in_=lns0[:], func=AF.Exp, bias=czero[:32], scale=0.5
                )
                den0 = spool.tile([32, OC], dt.float32, tag="den0")
                nc.vector.scalar_tensor_tensor(
                    out=den0[:], in0=ns0[:], scalar=1.0, in1=sqn0[:],
                    op0=ALU.add, op1=ALU.mult,
                )
                rden0 = spool.tile([32, OC], dt.float32, tag="rden0")
                nc.vector.reciprocal(out=rden0[:], in_=den0[:])
                fac0 = spool.tile([32, OC], dt.float32, tag="fac0")
                # fac = 0.1 * ns * rden  (0.1 for s0 = 0.1*T)
                nc.vector.tensor_tensor(
                    out=fac0[:], in0=ns0[:], in1=rden0[:], op=ALU.mult
                )
                nc.vector.tensor_scalar_mul(fac0[:], fac0[:], 0.1)
                v0_bf = spool.tile([32, ODF], dt.bfloat16, tag="v0_bf")
                nc.vector.tensor_tensor(
                    out=v0_bf[:].rearrange("p (o d) -> p o d", o=OC),
                    in0=ps0[:].rearrange("p (o d) -> p o d", o=OC),
                    in1=fac0[:].unsqueeze(2).broadcast_to([32, OC, OD]),
                    op=ALU.mult,
                )
                return v0_bf

            def s0_vreps(v0_bf):
                # vrep0 per bgroup via select matmul. Emitted AFTER some
                # einsum chunks: queueing these on PE right after the s0
                # matmuls stalled PE ~5us waiting on the squash chain.
                for bg in range(NBG):
                    pv = pv_pool.tile([128, ODF], dt.float32, tag="pv")
                    nc.tensor.matmul(
                        pv[:], sel[:, bg * 128 : (bg + 1) * 128], v0_bf[:],
                        start=True, stop=True,
                    )
                    vr = vpool.tile([128, ODF], dt.bfloat16, tag=f"vrep0{bg}")
                    nc.scalar.copy(out=vr[:], in_=pv[:])
                    vrep0_t.append(vr)

            # ---- Phase A: einsum for all bgroups
            uh_t = []
            bl_t = []
            for bg in range(NBG):
                uh = uhpool.tile([128, G * ODF], dt.bfloat16, tag=f"uh{bg}")
                uh_t.append(uh)
                bl = blpool.tile([128, GO], dt.bfloat16, tag=f"bl{bg}")
                bl_t.append(bl)

            def einsum_bg(bg, xcs=None):
                uh = uh_t[bg]
                for xc in xcs if xcs is not None else range(NXCH):
                    xt = xpool.tile([128, XCH * 128], dt.bfloat16, tag="xt")
                    nc.sync.dma_start(
                        out=xt[:],
                        in_=xblk_d[bg][:, xc * XCH * 128 : (xc + 1) * XCH * 128],
                    )
                    for t in range(XCH // 6):
                        pe = pe_pool.tile([128, 960], dt.float32, tag="pe")
                        for j in range(6):
                            gl = t * 6 + j           # local g in chunk
                            g = xc * XCH + gl        # global g
                            nc.tensor.matmul(
                                pe[:, j * ODF : (j + 1) * ODF],
                                xt[:, gl * 128 : (gl + 1) * 128],
                                wr_g(g),
                                start=True,
                                stop=True,
                            )
                        g0 = xc * XCH + t * 6
                        if bg == 0 and t % 2 == 1:
                            nc.vector.tensor_copy(
                                out=uh[:, g0 * ODF : (g0 + 6) * ODF], in_=pe[:]
                            )
                        else:
                            nc.scalar.copy(
                                out=uh[:, g0 * ODF : (g0 + 6) * ODF], in_=pe[:]
                            )

            GPS_CHUNK = False  # GpSimd bl-update chunk: measured far slower

            def bl_update(bg, vrep, first):
                """bl[bg] (+)= sum_d uh[bg]*vrep ; first=True writes fresh.
                Last chunk runs entirely on GpSimd in parallel with VectorE
                doing the other three."""
                bl = bl_t[bg]
                for ch in range(NZCH):
                    eng = nc.gpsimd if (GPS_CHUNK and ch == NZCH - 1) else nc.vector
                    tail = eng
                    cs = ch * ZCH
                    z = zpool.tile([128, ZCH * ODF], dt.bfloat16,
                                   tag="z")
                    eng.tensor_tensor(
                        out=z[:].rearrange("p (g f) -> p g f", f=ODF),
                        in0=uh_t[bg][:, cs * ODF : (cs + ZCH) * ODF].rearrange(
                            "p (g f) -> p g f", f=ODF
                        ),
                        in1=vrep[:].unsqueeze(1).broadcast_to([128, ZCH, ODF]),
                        op=ALU.mult,
                    )
                    t8 = zpool.tile([128, ZCH * 80], dt.bfloat16,
                                    tag="t8")
                    zv = z[:].rearrange("p (g o d) -> p g o d", o=OC, d=OD)
                    eng.tensor_tensor(
                        out=t8[:].rearrange("p (g o d) -> p g o d", o=OC, d=8),
                        in0=zv[:, :, :, 0:8],
                        in1=zv[:, :, :, 8:16],
                        op=ALU.add,
                    )
                    t4 = zpool.tile([128, ZCH * 40], dt.bfloat16,
                                    tag="t4")
                    t8v = t8[:].rearrange("p (g o d) -> p g o d", o=OC, d=8)
                    eng.tensor_tensor(
                        out=t4[:].rearrange("p (g o d) -> p g o d", o=OC, d=4),
                        in0=t8v[:, :, :, 0:4],
                        in1=t8v[:, :, :, 4:8],
                        op=ALU.add,
                    )
                    t2 = zpool.tile([128, ZCH * 20], dt.bfloat16,
                                    tag="t2")
                    t4v = t4[:].rearrange("p (g o d) -> p g o d", o=OC, d=4)
                    tail.tensor_tensor(
                        out=t2[:].rearrange("p (g o d) -> p g o d", o=OC, d=2),
                        in0=t4v[:, :, :, 0:2],
                        in1=t4v[:, :, :, 2:4],
                        op=ALU.add,
                    )
                    t2v = t2[:].rearrange("p (g o d) -> p g o d", o=OC, d=2)
                    blv = bl[:, cs * OC : (cs + ZCH) * OC]
                    if first:
                        tail.tensor_tensor(
                            out=blv.rearrange("p (g o) -> p g o", o=OC).unsqueeze(3),
                            in0=t2v[:, :, :, 0:1],
                            in1=t2v[:, :, :, 1:2],
                            op=ALU.add,
                        )
                    else:
                        t1 = zpool.tile([128, ZCH * OC], dt.bfloat16,
                                        tag="t1")
                        tail.tensor_tensor(
                            out=t1[:].rearrange("p (g o) -> p g o", o=OC).unsqueeze(3),
                            in0=t2v[:, :, :, 0:1],
                            in1=t2v[:, :, :, 1:2],
                            op=ALU.add,
                        )
                        tail.tensor_tensor(
                            out=blv, in0=blv, in1=t1[:], op=ALU.add
                        )

            # ---- iter-0: einsum interleaved with bl updates (software pipeline)
            # s0 FIRST: its dense matmul needs only x2d+wr (no xblk), so v0/
            # vrep0 are ready ~5us earlier and the DVE update chain starts
            # as soon as uh(0) lands.
            v0_bf = s0_phase()
            einsum_bg(0, xcs=(0, 1))
            s0_vreps(v0_bf)
            einsum_bg(0, xcs=(2, 3))
            einsum_bg(1)
            bl_update(0, vrep0_t[0], first=True)
            einsum_bg(2)
            bl_update(1, vrep0_t[1], first=True)
            einsum_bg(3)

            # ---- iters 1,2: stages, software-pipelined across (it,bg) columns
            col_state = {}

            # persistent ee per bg: it2 uses ee2 = exp(delta)*ee1 so the it1
            # bl-update can write its delta via the cheap first=True path
            # (no t1 + bl-add TTs on VectorE)
            ee1_t = [None] * NBG

            exp_done = {}

            def stage1_exp(it, bg):
                """Emit the column's Exp on ScalarE. Hoisted into the
                PREVIOUS column's emission so zz never head-blocks the DVE
                queue waiting on ScalarE at column start."""
                bl = bl_t[bg]
                if it == 1:
                    ee = eepool.tile([128, GO], dt.bfloat16, tag=f"ee1_{bg}")
                    ee1_t[bg] = ee
                    nc.scalar.activation(
                        out=ee[:], in_=bl[:], func=AF.Exp, bias=czero[:]
                    )
                    exp_done[(it, bg)] = ee
                else:
                    eeD = rpool.tile([128, GO], dt.bfloat16, tag="eeD")
                    nc.scalar.activation(
                        out=eeD[:], in_=bl[:], func=AF.Exp, bias=czero[:]
                    )
                    exp_done[(it, bg)] = eeD

            def stage1(it, bg, vcall_all=False):
                """softmax over o + call build. vcall_all: build every b'
                lane on VectorE (last column: keeps ScalarE off the exposed
                tail chain)."""
                if (it, bg) not in exp_done:
                    stage1_exp(it, bg)
                if it == 1:
                    ee = exp_done.pop((it, bg))
                else:
                    eeD = exp_done.pop((it, bg))
                    ee = rpool.tile([128, GO], dt.bfloat16, tag="ee")
                    nc.vector.tensor_tensor(
                        out=ee[:], in0=eeD[:], in1=ee1_t[bg][:], op=ALU.mult
                    )
                zz = rpool.tile([128, G], dt.float32, tag="zz")
                nc.vector.tensor_reduce(
                    out=zz[:],
                    in_=ee[:].rearrange("p (g o) -> p g o", o=OC),
                    axis=AX.X,
                    op=ALU.add,
                )
                rz = rpool.tile([128, G], dt.float32, tag="rz")
                nc.vector.reciprocal_approx_fast(out=rz[:], in_=zz[:])
                # cCb = ee * (1/Z) in ONE TT with broadcast in1 (runs 1x, but
                # replaces the old rzb CAST@1x + cCb TT@2x pair)
                cCb = rpool.tile([128, GO], dt.bfloat16, tag="cCb")
                nc.vector.tensor_tensor(
                    out=cCb[:].rearrange("p (g o) -> p g o", o=OC),
                    in0=ee[:].rearrange("p (g o) -> p g o", o=OC),
                    in1=rz[:].unsqueeze(2).broadcast_to([128, G, OC]),
                    op=ALU.mult,
                )
                call = rpool.tile([128, G * 80], dt.bfloat16, tag="call")
                callv = call[:].rearrange(
                    "p (g b o) -> p g b o", b=BG, o=OC
                )
                # call build: split 2 lanes VectorE tensor_scalar / 6 ScalarE
                # masked copies (all-Act delays the s-matmul chain; all-DVE
                # loads the binding engine). vcall_all (last column): all on
                # VectorE so no ScalarE hop sits on the exposed tail chain.
                nv = BG if vcall_all else 2
                cCv = cCb[:].rearrange("p (g o) -> p g o", o=OC)
                for bp in range(nv):
                    nc.vector.tensor_scalar_mul(
                        callv[:, :, bp, :], cCv, mcf[:, bp * OC : bp * OC + 1]
                    )
                for bp in range(nv, BG):
                    nc.scalar.activation(
                        out=callv[:, :, bp, :],
                        in_=cCb[:].rearrange("p (g o) -> p g o", o=OC),
                        func=AF.Copy,
                        bias=0.0,
                        scale=mcf[:, bp * OC : bp * OC + 1],
                    )
                col_state[(it, bg)] = {"call": call}

            def stage2(it, bg, split=False):
                """s matmul: accumulate over all 72 groups."""
                call = col_state[(it, bg)]["call"]
                ps = ps_pool.tile([80, ODF], dt.float32, tag="ps")
                for g in range(G):
                    nc.tensor.matmul(
                        ps[:],
                        call[:, g * 80 : (g + 1) * 80],
                        uh_t[bg][:, g * ODF : (g + 1) * ODF],
                        start=(g == 0),
                        stop=(g == G - 1),
                    )
                col_state[(it, bg)]["ps"] = ps

            def stage3(it, bg, last=False):
                """diag extract + squash; it1: build vrep + bl update input,
                it2: final v -> DRAM. last: the tail column - keep the chain
                off ScalarE where DVE is idle anyway."""
                # extract diag o==o' -> s_t [80,16]
                tmp = spool.tile([80, ODF], dt.float32, tag="tmp")
                mskv = (
                    msks[:]
                    .rearrange("p (o d) -> p o d", o=OC)
                    .transpose([0, 2, 1])
                )
                if "ps2" in col_state[(it, bg)]:
                    ps2 = col_state[(it, bg)]["ps2"]
                    tmh = col_state[(it, bg)]["tmh"]
                    tm2 = cpool.tile([80, ODF], dt.float32, tag="tm2")
                    nc.vector.tensor_tensor(
                        out=tm2[:].rearrange("p (d o) -> p d o", o=OC),
                        in0=ps2[:, ODF : 2 * ODF]
                        .rearrange("p (o d) -> p o d", o=OC)
                        .transpose([0, 2, 1]),
                        in1=mskv,
                        op=ALU.mult,
                    )
                    nc.vector.tensor_tensor(
                        out=tmp[:], in0=tmh[:], in1=tm2[:], op=ALU.add
                    )
                else:
                    ps = col_state[(it, bg)]["ps"]
                    nc.vector.tensor_tensor(
                        out=tmp[:].rearrange("p (d o) -> p d o", o=OC),
                        in0=ps[:]
                        .rearrange("p (o d) -> p o d", o=OC)
                        .transpose([0, 2, 1]),
                        in1=mskv,
                        op=ALU.mult,
                    )
                s_t = spool.tile([80, OD], dt.float32, tag="s_t")
                nc.vector.tensor_reduce(
                    out=s_t[:],
                    in_=tmp[:].rearrange("p (d o) -> p d o", o=OC),
                    axis=AX.X,
                    op=ALU.add,
                )

                # squash: fac = ns / ((1+ns) * sqrt(ns+eps))
                # (tried Square on DVE for the last column: measured ~0.8us
                # WORSE - keep it on ScalarE)
                sq = spool.tile([80, OD], dt.float32, tag="sq")
                ns = spool.tile([80, 1], dt.float32, tag="ns")
                nc.scalar.activation(
                    out=sq[:], in_=s_t[:], func=AF.Square, bias=czero[:80]
                )
                nc.vector.tensor_reduce(
                    out=ns[:], in_=sq[:], axis=AX.X, op=ALU.add
                )
                lns = spool.tile([80, 1], dt.float32, tag="lns")
                nc.scalar.activation(
                    out=lns[:], in_=ns[:], func=AF.Ln, bias=ceps[:]
                )
                sqn = spool.tile([80, 1], dt.float32, tag="sqn")
                nc.scalar.activation(
                    out=sqn[:], in_=lns[:], func=AF.Exp, bias=czero[:80], scale=0.5
                )
                den = spool.tile([80, 1], dt.float32, tag="den")
                nc.vector.scalar_tensor_tensor(
                    out=den[:], in0=ns[:], scalar=1.0, in1=sqn[:],
                    op0=ALU.add, op1=ALU.mult,
                )
                rden = spool.tile([80, 1], dt.float32, tag="rden")
                nc.vector.reciprocal(out=rden[:], in_=den[:])
                fac = spool.tile([80, 1], dt.float32, tag="fac")
                nc.vector.tensor_tensor(
                    out=fac[:], in0=ns[:], in1=rden[:], op=ALU.mult
                )

                if it == ITERS - 1:
                    v_f = spool.tile([80, OD], dt.float32, tag="v_f")
                    nc.vector.tensor_scalar_mul(v_f[:], s_t[:], fac[:])
                    nc.sync.dma_start(
                        out=vout_d[bg * BG : (bg + 1) * BG].rearrange(
                            "b o d -> (b o) d"
                        ),
                        in_=v_f[:],
                    )
                    return

                v_bf = spool.tile([80, OD], dt.bfloat16, tag="v_bf")
                nc.vector.tensor_scalar_mul(v_bf[:], s_t[:], fac[:])

                # vexp[(b,o),(o',d)] = v[b,o,d] * delta(o==o')
                vexp = spool.tile([80, ODF], dt.bfloat16, tag="vexp")
                nc.vector.tensor_tensor(
                    out=vexp[:].rearrange("p (o d) -> p o d", o=OC),
                    in0=msks[:].rearrange("p (o d) -> p o d", o=OC),
                    in1=v_bf[:].unsqueeze(1).broadcast_to([80, OC, OD]),
                    op=ALU.mult,
                )
                pv = pv_pool.tile([128, ODF], dt.float32, tag="pv")
                nc.tensor.matmul(pv[:], arep[:], vexp[:], start=True, stop=True)
                vrep = vpool.tile([128, ODF], dt.bfloat16, tag="vrep")
                nc.scalar.copy(out=vrep[:], in_=pv[:])
                col_state[(it, bg)]["vrep"] = vrep

            # pipelined emission: stage1(n) | stage3(n-1) | stage2(n) |
            # bl_update(n-1, it1 only)
            # seam: start the it1 pipeline for bg0 between iter-0 updates so
            # ScalarE's exp is queued before the last iter-0 bl-updates
            stage1(1, 0)
            stage1_exp(1, 1)
            bl_update(2, vrep0_t[2], first=True)
            stage2(1, 0)
            bl_update(3, vrep0_t[3], first=True)

            # it2 ends (2,3),(2,2): the final column's bl (bg2) is updated a
            # slot earlier, and the last two s-matmuls run back-to-back on a
            # warm PE while stage3(2,3) overlaps s-matmul(2,2).
            cols = [(1, 0), (1, 1), (1, 2), (1, 3), (2, 0), (2, 1), (2, 3), (2, 2)]
            # bl-update lags TWO columns so VectorE never waits on the
            # in-flight s-matmul; bg2's update stays last so VectorE
            # stays busy during the final it2 s-matmuls
            upd_at = {2: 0, 3: 1, 5: 3, 6: 2}
            for n in range(1, len(cols)):
                stage1(*cols[n], vcall_all=(n == len(cols) - 1))
                if n in upd_at:
                    c = cols[upd_at[n]]
                    # first=True: write the it1 delta over bl (dead after
                    # ee1 = exp(bl) was taken); it2 recombines via ee1*exp(d)
                    bl_update(c[1], col_state[c]["vrep"], first=True)
                if n + 1 < len(cols):
                    # hoist next column's Exp (legal: its bl-update, if any,
                    # was just emitted at this slot)
                    stage1_exp(*cols[n + 1])
                stage3(*cols[n - 1])
                stage2(*cols[n])
            stage3(*cols[-1], last=True)

    nc.finalize()
    _BUILT = nc
    return nc


_WARMED = False


def kernel(x, W):
    global _WARMED
    nc = _build()
    in_maps = _in_maps(x, W)
    if not _WARMED:
        # First execution after an in-process compile can return a
        # partially-unwritten output buffer (observed: bgroup 0 rows NaN).
        # Run once with the real inputs and discard.
        run_bass_kernel_spmd(nc, in_maps, core_ids=list(range(NCORES)))
        _WARMED = True
    res = run_bass_kernel_spmd(nc, in_maps, core_ids=list(range(NCORES)))
    outs = res.results
    v = np.concatenate([np.asarray(o["vout"]) for o in outs], axis=0)
    return v.astype(np.float32)


if __name__ == "__main__":
    rng = np.random.default_rng(0)
    x = rng.standard_normal((B, IC, KD), np.float32)
    W = rng.standard_normal((IC, OC, OD, KD), np.float32)
    v = kernel(x, W)
    print("out", v.shape, v.dtype, float(np.abs(v).mean()))

